# revision 33
# baseline (speedup 1.0000x reference)
"""MultiHeadAttention Trainium2 Bass kernel.

Head-sharded tensor parallel across 8 NeuronCores (2 heads/core).
All-transposed dataflow: activations live feature-on-partition so no
on-device activation transposes are needed; the per-head attention
computes S.T = K Q.T directly, softmax is max-free (scores are bounded),
the additive attention bias is applied as a multiply by exp(bias)
(precomputed on host), and the key-padding mask is applied by zeroing
masked v rows + masking the denominator matmul.

Host side: inputs are pre-transposed / pre-cast to fp16, outputs are
partial sums (row-parallel out projection) summed on host.
"""

import sys

sys.path.insert(0, "/opt/trn_rl_repo")

import numpy as np

B, S, H, NH = 2, 2048, 1024, 16
HD = H // NH            # 64
NCORES = 8
HPC = NH // NCORES      # 2 heads per core
CW = HPC * HD           # 128 = per-core slice width
R = B * S               # 4096 flattened rows
SCALE = float(HD) ** -0.5
F = H // 128            # 8 feature blocks
RC = R // 512           # 8 row chunks
QC = S // 512           # 4 q chunks per batch
KB = S // 128           # 16 k blocks per batch
T = B * KB              # 32 (b, kb) blocks

_CACHE = {}


def _build_module():
    import concourse.bass as bass
    import concourse.tile as tile
    from concourse import bacc, mybir
    from concourse.masks import make_identity

    f16 = mybir.dt.float16
    f32 = mybir.dt.float32
    Exp = mybir.ActivationFunctionType.Exp

    nc = bacc.Bacc(
        "TRN2", target_bir_lowering=False, debug=False, num_devices=NCORES
    )

    # ---- DRAM I/O (per core) ----
    xq = nc.dram_tensor("xq_t", [H, R], f16, kind="ExternalInput").ap()
    xk = nc.dram_tensor("xk_t", [H, R], f16, kind="ExternalInput").ap()
    xv = nc.dram_tensor("xv_t", [H, R], f16, kind="ExternalInput").ap()
    wq = nc.dram_tensor("wq_t", [H, CW], f16, kind="ExternalInput").ap()
    wk = nc.dram_tensor("wk_t", [H, CW], f16, kind="ExternalInput").ap()
    wv = nc.dram_tensor("wv_t", [H, CW], f16, kind="ExternalInput").ap()
    wo = nc.dram_tensor("wo_t", [CW, H], f16, kind="ExternalInput").ap()
    qb = nc.dram_tensor("qb_col", [CW, 1], f32, kind="ExternalInput").ap()
    kb_ = nc.dram_tensor("kb_col", [CW, 1], f32, kind="ExternalInput").ap()
    eb = nc.dram_tensor("eb_t", [QC, S, HPC * 512], f16,
                        kind="ExternalInput").ap()
    m01f = nc.dram_tensor("m01_f32", [128, T], f32, kind="ExternalInput").ap()
    m01h = nc.dram_tensor("m01_v", [128, T], f16, kind="ExternalInput").ap()
    opart = nc.dram_tensor("o_part", [R, H], f16, kind="ExternalOutput").ap()

    with tile.TileContext(nc) as tc:
        _emit(tc, nc, f16, f32, Exp, make_identity, bass,
              xq, xk, xv, wq, wk, wv, wo, qb, kb_, eb, m01f, m01h, opart)

    nc.compile()
    return nc


def _emit(tc, nc, f16, f32, Exp, make_identity, bass,
          xq, xk, xv, wq, wk, wv, wo, qb, kb_, eb, m01f, m01h, opart):
    from contextlib import ExitStack

    with ExitStack() as top:
        consts = top.enter_context(tc.tile_pool(name="consts", bufs=1))
        pers = top.enter_context(tc.tile_pool(name="pers", bufs=1))
        xpool = top.enter_context(tc.tile_pool(name="xin", bufs=4))

        # ---- constants / weights resident in SBUF ----
        wq_sb = consts.tile([128, F, 128], f16, tag="wq")
        nc.sync.dma_start(wq_sb, wq.rearrange("(f p) j -> p f j", p=128))
        wk_sb = consts.tile([128, F, 128], f16, tag="wk")
        nc.sync.dma_start(wk_sb, wk.rearrange("(f p) j -> p f j", p=128))
        wv_sb = consts.tile([128, F, 128], f16, tag="wv")
        nc.sync.dma_start(wv_sb, wv.rearrange("(f p) j -> p f j", p=128))
        wo_sb = consts.tile([128, H], f16, tag="wo")
        nc.sync.dma_start(wo_sb, wo)
        qb_sb = consts.tile([128, 1], f32, tag="qb")
        nc.sync.dma_start(qb_sb, qb)
        kb_sb = consts.tile([128, 1], f32, tag="kb")
        nc.sync.dma_start(kb_sb, kb_)
        m01f_sb = consts.tile([128, T], f32, tag="m01f")
        nc.sync.dma_start(m01f_sb, m01f)
        ident = consts.tile([128, 128], f16, tag="ident")
        make_identity(nc, ident)

        # ---- persistent activations ----
        qT_sb = pers.tile([128, R], f16, tag="qT")     # [2h*64d, (b,s)]
        kT_sb = pers.tile([128, R], f16, tag="kT")
        # v_aug layout per (b,kb) block t: [v_h0 (0:64) | m01 (64) | pad |
        #                                   v_h1 (66:130) | m01 (130) | pad]
        v_nat = pers.tile([128, T, 132], f16, tag="vn")
        # fill the mask columns (the PV "ones column" → masked denominator)
        nc.sync.dma_start(v_nat[:, :, 64:65], m01h)
        nc.sync.dma_start(v_nat[:, :, 130:131], m01h)
        ctxn = [pers.tile([128, S], f16, tag=f"ctxn{b}", name=f"ctxn{b}")
                for b in range(B)]

        # =================== phase 1: projections ===================
        with tc.tile_pool(name="p1psum", bufs=4, space="PSUM") as p1, \
             tc.tile_pool(name="ptrans", bufs=3, space="PSUM") as ptr, \
             tc.tile_pool(name="vt", bufs=2) as vtp:

            for w_sb, x_dram, dst, bias_col in (
                (wq_sb, xq, qT_sb, qb_sb),
                (wk_sb, xk, kT_sb, kb_sb),
            ):
                xr = x_dram.rearrange("(f p) r -> p f r", p=128)
                for rc in range(RC):
                    xt = xpool.tile([128, F, 512], f16, tag="xt")
                    nc.sync.dma_start(xt, xr[:, :, rc * 512:(rc + 1) * 512])
                    ps = p1.tile([128, 512], f32, tag="p1")
                    for f in range(F):
                        nc.tensor.matmul(ps, lhsT=w_sb[:, f, :],
                                         rhs=xt[:, f, :],
                                         start=(f == 0), stop=(f == F - 1))
                    nc.vector.tensor_scalar_add(
                        dst[:, rc * 512:(rc + 1) * 512], ps, bias_col)

            # v: project (v.T chunks), then PE-transpose to natural layout,
            # zeroing masked key rows via the 0/1 mask column.
            xvr = xv.rearrange("(f p) r -> p f r", p=128)
            for rc in range(RC):
                xt = xpool.tile([128, F, 512], f16, tag="xt")
                nc.sync.dma_start(xt, xvr[:, :, rc * 512:(rc + 1) * 512])
                ps = p1.tile([128, 512], f32, tag="p1")
                for f in range(F):
                    nc.tensor.matmul(ps, lhsT=wv_sb[:, f, :], rhs=xt[:, f, :],
                                     start=(f == 0), stop=(f == F - 1))
                vt = vtp.tile([128, 512], f16, tag="vt")
                nc.vector.tensor_copy(vt, ps)
                for i in range(4):
                    t = rc * 4 + i          # t = b*KB + kb
                    col = (t % KB) * B + t // KB
                    tp = ptr.tile([128, 128], f16, tag="tp")
                    nc.tensor.transpose(tp, vt[:, i * 128:(i + 1) * 128], ident)
                    for h in range(HPC):
                        nc.vector.tensor_scalar_mul(
                            v_nat[:, t, h * 66:h * 66 + 64],
                            tp[:, h * 64:(h + 1) * 64],
                            m01f_sb[:, col:col + 1])

        # =================== phase 2: attention ===================
        with tc.tile_pool(name="qkpsum", bufs=3, space="PSUM") as qkp, \
             tc.tile_pool(name="cvpsum", bufs=2, space="PSUM") as cvp_pool, \
             tc.tile_pool(name="ebp", bufs=2) as ebp, \
             tc.tile_pool(name="esp", bufs=4) as esp, \
             tc.tile_pool(name="ptp", bufs=4) as ptp, \
             tc.tile_pool(name="bcp", bufs=2) as bcp, \
             tc.tile_pool(name="rcp", bufs=2) as rcp, \
             tc.tile_pool(name="dscr", bufs=4, space="DRAM") as dscr:

            ebr = eb.rearrange("qc (kb p) m -> p qc kb m", p=128)
            ctx1 = [pers.tile([64, S], f16, tag=f"ctx1{b}", name=f"ctx1{b}")
                    for b in range(B)]
            for qc in range(QC):
                # whole-qc EB block resident: reused by both batches
                ebq = ebp.tile([128, KB, HPC * 512], f16, tag="eb")
                for g in range(4):
                    nc.sync.dma_start(ebq[:, g * 4:(g + 1) * 4, :],
                                      ebr[:, qc, g * 4:(g + 1) * 4, :])

                rc_sb = rcp.tile([65, B * HPC, 512], f32, tag="rc")
                for b in range(B):
                    cvp = [cvp_pool.tile([65, 512], f32, tag="cv",
                                         name=f"cv{qc}_{b}_{h}")
                           for h in range(HPC)]
                    for kb in range(KB):
                        sps = qkp.tile([128, HPC, 512], f32, tag="sps",
                                       name=f"sps{qc}_{kb}_{b}")
                        for h in range(HPC):
                            nc.tensor.matmul(
                                sps[:, h, :],
                                lhsT=kT_sb[h * 64:(h + 1) * 64,
                                           b * S + kb * 128:
                                           b * S + (kb + 1) * 128],
                                rhs=qT_sb[h * 64:(h + 1) * 64,
                                          b * S + qc * 512:
                                          b * S + (qc + 1) * 512],
                                start=True, stop=True)
                        est = esp.tile([128, HPC, 512], f16, tag="es")
                        nc.scalar.activation(est, sps, func=Exp, scale=SCALE)
                        ptt = ptp.tile([128, HPC, 512], f16, tag="pt")
                        ebt = ebq[:, kb, :].rearrange("p (i q) -> p i q",
                                                      i=HPC)
                        eng = nc.gpsimd if kb % 3 == 2 else nc.vector
                        eng.tensor_mul(ptt, est, ebt)

                        for h in range(HPC):
                            # v_aug lhsT: 64 v cols + the 0/1 mask column →
                            # rows 0-63 = ctx.T, row 64 = masked denominator
                            nc.tensor.matmul(
                                cvp[h],
                                lhsT=v_nat[:, b * KB + kb,
                                           h * 66:h * 66 + 65],
                                rhs=ptt[:, h, :],
                                start=(kb == 0), stop=(kb == KB - 1))

                    # normalize: ctxn = ctx.T * (1/den), per h
                    for h in range(HPC):
                        i = b * HPC + h
                        nc.vector.reciprocal(rc_sb[64:65, i, :],
                                             cvp[h][64:65, :])
                    scr = dscr.tile([1, HPC, 512], f32, tag="scr",
                                    name=f"scr{qc}_{b}")
                    nc.scalar.dma_start(scr,
                                        rc_sb[64:65,
                                              b * HPC:(b + 1) * HPC, :])
                    bc = bcp.tile([64, HPC, 512], f32, tag="bc")
                    nc.scalar.dma_start(bc,
                                        scr.to_broadcast((64, HPC, 512)))
                    nc.vector.tensor_mul(
                        ctxn[b][0:64, qc * 512:(qc + 1) * 512],
                        cvp[0][0:64, :], bc[:, 0, :])
                    # h1: lanes 0-63; gathered in ctx1, relocated to
                    # ctxn partitions 64-127 once after the qc loop
                    nc.vector.tensor_mul(
                        ctx1[b][:, qc * 512:(qc + 1) * 512],
                        cvp[1][0:64, :], bc[:, 1, :])
            for b in range(B):
                nc.scalar.dma_start(ctxn[b][64:128, :], ctx1[b])

        # =================== phase 3: out projection ===================
        OB = 4                          # row-blocks per output DMA
        opr = opart.rearrange("(g p) hh -> p g hh", p=128)
        with tc.tile_pool(name="p3psum", bufs=2, space="PSUM") as p3, \
             tc.tile_pool(name="op", bufs=2) as op:
            for b in range(B):
                for rb in range(S // 128):
                    g = b * (S // 128) + rb
                    if rb % OB == 0:
                        ob_g = op.tile([128, OB, H], f16, tag="ob")
                    po = p3.tile([128, 2, 512], f32, tag="po")
                    lhsT = ctxn[b][:, rb * 128:(rb + 1) * 128]
                    nc.tensor.matmul(po[:, 0, :], lhsT=lhsT,
                                     rhs=wo_sb[:, 0:512], start=True, stop=True)
                    nc.tensor.matmul(po[:, 1, :], lhsT=lhsT,
                                     rhs=wo_sb[:, 512:1024], start=True,
                                     stop=True)
                    dstv = ob_g[:, rb % OB, :].rearrange("p (i j) -> p i j",
                                                         i=2)
                    if rb % 2 == 0:
                        nc.vector.tensor_copy(dstv, po)
                    else:
                        nc.scalar.copy(dstv, po)
                    if rb % OB == OB - 1:
                        nc.sync.dma_start(opr[:, g - (OB - 1):g + 1, :], ob_g)


def get_module():
    if "nc" not in _CACHE:
        _CACHE["nc"] = _build_module()
    return _CACHE["nc"]


def make_in_maps(query, key, value, key_padding_mask, bias,
                 q_w, q_b, k_w, k_b, v_w, v_b, o_w, o_b):
    f16 = np.float16
    xq_t = np.ascontiguousarray(query.reshape(R, H).T).astype(f16)
    xk_t = np.ascontiguousarray(key.reshape(R, H).T).astype(f16)
    xv_t = np.ascontiguousarray(value.reshape(R, H).T).astype(f16)

    kpm = np.asarray(key_padding_mask)
    # m01[p, b*?]: column index kb*B + b ; 0.0 where masked
    m01 = np.empty((128, T), np.float32)
    for b in range(B):
        for kb in range(KB):
            m01[:, kb * B + b] = np.where(kpm[b, kb * 128:(kb + 1) * 128],
                                          0.0, 1.0)
    m01_f32 = np.ascontiguousarray(m01)
    # v-order mask: column t = b*KB + kb (matches the v_nat block order)
    m01v = np.empty((128, T), f16)
    for b in range(B):
        for kb in range(KB):
            m01v[:, b * KB + kb] = m01[:, kb * B + b].astype(f16)

    in_maps = []
    for c in range(NCORES):
        hs = slice(c * CW, (c + 1) * CW)
        # eb layout [qc, k, i, qi]: exp(bias).T pre-sliced by q chunk
        ebt = np.empty((QC, S, HPC, 512), f16)
        for i in range(HPC):
            h = c * HPC + i
            e = np.exp(np.asarray(bias[0, h], np.float32).T).astype(f16)
            ebt[:, :, i, :] = e.reshape(S, QC, 512).transpose(1, 0, 2)
        ebt = ebt.reshape(QC, S, HPC * 512)
        in_maps.append({
            "xq_t": xq_t, "xk_t": xk_t, "xv_t": xv_t,
            "wq_t": np.ascontiguousarray(np.asarray(q_w)[hs].T).astype(f16),
            "wk_t": np.ascontiguousarray(np.asarray(k_w)[hs].T).astype(f16),
            "wv_t": np.ascontiguousarray(np.asarray(v_w)[hs].T).astype(f16),
            "wo_t": np.ascontiguousarray(np.asarray(o_w)[:, hs].T).astype(f16),
            "qb_col": np.asarray(q_b, np.float32)[hs].reshape(CW, 1).copy(),
            "kb_col": np.asarray(k_b, np.float32)[hs].reshape(CW, 1).copy(),
            "eb_t": ebt,
            "m01_f32": m01_f32,
            "m01_v": m01v,
        })
    return in_maps


def assemble_output(results, v_b, o_w, o_b):
    acc = np.zeros((R, H), np.float32)
    for res in results:
        acc += np.asarray(res["o_part"], np.float32)
    corr = np.asarray(v_b, np.float32) @ np.asarray(o_w, np.float32).T \
        + np.asarray(o_b, np.float32)
    acc += corr[None, :]
    return acc.reshape(B, S, H).astype(np.float32)


def kernel(**inputs):
    from concourse.bass_utils import run_bass_kernel_spmd

    nc = get_module()
    in_maps = make_in_maps(**inputs)
    res = run_bass_kernel_spmd(nc, in_maps, list(range(NCORES)))
    return assemble_output(res.results, inputs["v_b"], inputs["o_w"],
                           inputs["o_b"])


# revision 47
# speedup vs baseline: 1.3440x; 1.3440x over previous
"""MultiHeadAttention Trainium2 Bass kernel.

Head-sharded tensor parallel across 8 NeuronCores (2 heads/core).
All-transposed dataflow: activations live feature-on-partition so no
on-device activation transposes are needed; the per-head attention
computes S.T = K Q.T directly, softmax is max-free (scores are bounded),
the additive attention bias is applied as a multiply by exp(bias)
(precomputed on host), and the key-padding mask is applied by zeroing
masked v rows + masking the denominator matmul.

Host side: inputs are pre-transposed / pre-cast to fp16, outputs are
partial sums (row-parallel out projection) summed on host.
"""

import sys

sys.path.insert(0, "/opt/trn_rl_repo")

import numpy as np

B, S, H, NH = 2, 2048, 1024, 16
HD = H // NH            # 64
NCORES = 8
HPC = NH // NCORES      # 2 heads per core
CW = HPC * HD           # 128 = per-core slice width
R = B * S               # 4096 flattened rows
SCALE = float(HD) ** -0.5
F = H // 128            # 8 feature blocks
RC = R // 512           # 8 row chunks
QC = S // 512           # 4 q chunks per batch
KB = S // 128           # 16 k blocks per batch
T = B * KB              # 32 (b, kb) blocks

_CACHE = {}


def _build_module():
    import concourse.bass as bass
    import concourse.tile as tile
    from concourse import bacc, mybir
    from concourse.masks import make_identity

    f16 = mybir.dt.float16
    f32 = mybir.dt.float32
    Exp = mybir.ActivationFunctionType.Exp

    nc = bacc.Bacc(
        "TRN2", target_bir_lowering=False, debug=False, num_devices=NCORES
    )

    # ---- DRAM I/O (per core) ----
    xq = nc.dram_tensor("xq_t", [H, R], f16, kind="ExternalInput").ap()
    xk = nc.dram_tensor("xk_t", [H, R], f16, kind="ExternalInput").ap()
    xv = nc.dram_tensor("xv_t", [H, R], f16, kind="ExternalInput").ap()
    wq = nc.dram_tensor("wq_t", [H, CW], f16, kind="ExternalInput").ap()
    wk = nc.dram_tensor("wk_t", [H, CW], f16, kind="ExternalInput").ap()
    wv = nc.dram_tensor("wv_t", [H, CW], f16, kind="ExternalInput").ap()
    wo = nc.dram_tensor("wo_t", [CW, H], f16, kind="ExternalInput").ap()
    qb = nc.dram_tensor("qb_col", [CW, 1], f32, kind="ExternalInput").ap()
    kb_ = nc.dram_tensor("kb_col", [CW, 1], f32, kind="ExternalInput").ap()
    eb = nc.dram_tensor("eb_t", [QC, S, HPC * 512], f16,
                        kind="ExternalInput").ap()
    m01f = nc.dram_tensor("m01_f32", [128, T], f32, kind="ExternalInput").ap()
    m01h = nc.dram_tensor("m01_v", [128, T], f16, kind="ExternalInput").ap()
    opart = nc.dram_tensor("o_part", [R, H], f16, kind="ExternalOutput").ap()

    with tile.TileContext(nc) as tc:
        _emit(tc, nc, f16, f32, Exp, make_identity, bass,
              xq, xk, xv, wq, wk, wv, wo, qb, kb_, eb, m01f, m01h, opart)

    nc.compile()
    return nc


def _emit(tc, nc, f16, f32, Exp, make_identity, bass,
          xq, xk, xv, wq, wk, wv, wo, qb, kb_, eb, m01f, m01h, opart):
    from contextlib import ExitStack

    with ExitStack() as top:
        consts = top.enter_context(tc.tile_pool(name="consts", bufs=1))
        pers = top.enter_context(tc.tile_pool(name="pers", bufs=1))
        xpool = top.enter_context(tc.tile_pool(name="xin", bufs=4))

        # ---- tiles for constants / persistent activations ----
        wq_sb = consts.tile([128, F, 128], f16, tag="wq")
        wk_sb = consts.tile([128, F, 128], f16, tag="wk")
        wv_sb = consts.tile([128, F, 128], f16, tag="wv")
        wo_sb = consts.tile([128, H], f16, tag="wo")
        qb_sb = consts.tile([128, 1], f32, tag="qb")
        kb_sb = consts.tile([128, 1], f32, tag="kb")
        m01f_sb = consts.tile([128, T], f32, tag="m01f")
        ident = consts.tile([128, 128], f16, tag="ident")

        qT_sb = pers.tile([128, R], f16, tag="qT")     # [2h*64d, (b,s)]
        kT_sb = pers.tile([128, R], f16, tag="kT")
        # v_aug layout per (b,kb) block t: [v_h0 (0:64) | m01 (64) | pad |
        #                                   v_h1 (66:130) | m01 (130) | pad]
        v_nat = pers.tile([128, T, 132], f16, tag="vn")
        ctxn = [pers.tile([128, S], f16, tag=f"ctxn{b}", name=f"ctxn{b}")
                for b in range(B)]

        # =================== phase 1: projections ===================
        # Const DMAs are interleaved so the first q-proj matmul only waits
        # on wq + its first x tile, not the whole constant set.
        with tc.tile_pool(name="p1psum", bufs=4, space="PSUM") as p1, \
             tc.tile_pool(name="ptrans", bufs=3, space="PSUM") as ptr, \
             tc.tile_pool(name="vt", bufs=2) as vtp:

            for w_sb, w_dram, x_dram, dst, bias_col, b_dram in (
                (wq_sb, wq, xq, qT_sb, qb_sb, qb),
                (wk_sb, wk, xk, kT_sb, kb_sb, kb_),
            ):
                nc.sync.dma_start(w_sb,
                                  w_dram.rearrange("(f p) j -> p f j", p=128))
                nc.sync.dma_start(bias_col, b_dram)
                xr = x_dram.rearrange("(f p) r -> p f r", p=128)
                for rc in range(RC):
                    xt = xpool.tile([128, F, 512], f16, tag="xt")
                    nc.sync.dma_start(xt, xr[:, :, rc * 512:(rc + 1) * 512])
                    ps = p1.tile([128, 512], f32, tag="p1")
                    for f in range(F):
                        nc.tensor.matmul(ps, lhsT=w_sb[:, f, :],
                                         rhs=xt[:, f, :],
                                         start=(f == 0), stop=(f == F - 1))
                    nc.vector.tensor_scalar_add(
                        dst[:, rc * 512:(rc + 1) * 512], ps, bias_col)
                if w_sb is wq_sb:
                    # deferred consts, loaded in the shadow of q-proj
                    nc.sync.dma_start(
                        wv_sb, wv.rearrange("(f p) j -> p f j", p=128))
                    nc.sync.dma_start(m01f_sb, m01f)
                    make_identity(nc, ident)
                    nc.sync.dma_start(v_nat[:, :, 64:65], m01h)
                    nc.sync.dma_start(v_nat[:, :, 130:131], m01h)
                    nc.sync.dma_start(wo_sb, wo)

            # v: project (v.T chunks), then PE-transpose to natural layout,
            # zeroing masked key rows via the 0/1 mask column.
            xvr = xv.rearrange("(f p) r -> p f r", p=128)
            for rc in range(RC):
                xt = xpool.tile([128, F, 512], f16, tag="xt")
                nc.sync.dma_start(xt, xvr[:, :, rc * 512:(rc + 1) * 512])
                ps = p1.tile([128, 512], f32, tag="p1")
                for f in range(F):
                    nc.tensor.matmul(ps, lhsT=wv_sb[:, f, :], rhs=xt[:, f, :],
                                     start=(f == 0), stop=(f == F - 1))
                vt = vtp.tile([128, 512], f16, tag="vt")
                nc.vector.tensor_copy(vt, ps)
                for i in range(4):
                    t = rc * 4 + i          # t = b*KB + kb
                    col = (t % KB) * B + t // KB
                    tp = ptr.tile([128, 128], f16, tag="tp")
                    nc.tensor.transpose(tp, vt[:, i * 128:(i + 1) * 128], ident)
                    for h in range(HPC):
                        nc.vector.tensor_scalar_mul(
                            v_nat[:, t, h * 66:h * 66 + 64],
                            tp[:, h * 64:(h + 1) * 64],
                            m01f_sb[:, col:col + 1])

        # =================== phase 2: attention ===================
        with tc.tile_pool(name="qkpsum", bufs=3, space="PSUM") as qkp, \
             tc.tile_pool(name="cvpsum", bufs=2, space="PSUM") as cvp_pool, \
             tc.tile_pool(name="ebp", bufs=2) as ebp, \
             tc.tile_pool(name="esp", bufs=4) as esp, \
             tc.tile_pool(name="ptp", bufs=4) as ptp, \
             tc.tile_pool(name="bcp", bufs=2) as bcp, \
             tc.tile_pool(name="rcp", bufs=2) as rcp, \
             tc.tile_pool(name="op", bufs=2) as op, \
             tc.tile_pool(name="dscr", bufs=4, space="DRAM") as dscr:

            opr = opart.rearrange("(g p) hh -> p g hh", p=128)
            ebr = eb.rearrange("qc (kb p) m -> p qc kb m", p=128)
            ctx1 = [pers.tile([64, S], f16, tag=f"ctx1{b}", name=f"ctx1{b}")
                    for b in range(B)]
            PIPE = 2                    # PV trails QK by this many kb
            op_pend = []                # delayed out-projection emitters
            for qc in range(QC):
                # whole-qc EB block resident: reused by both batches
                ebq = ebp.tile([128, KB, HPC * 512], f16, tag="eb")
                for g in range(4):
                    nc.sync.dma_start(ebq[:, g * 4:(g + 1) * 4, :],
                                      ebr[:, qc, g * 4:(g + 1) * 4, :])

                for b in range(B):
                    cvp = [cvp_pool.tile([65, 512], f32, tag="cv",
                                         name=f"cv{qc}_{b}_{h}")
                           for h in range(HPC)]

                    def emit_pv(ptt, kb, b=b, cvp=cvp):
                        for h in range(HPC):
                            # v_aug lhsT: 64 v cols + 0/1 mask column →
                            # rows 0-63 = ctx.T, row 64 = masked denom
                            nc.tensor.matmul(
                                cvp[h],
                                lhsT=v_nat[:, b * KB + kb,
                                           h * 66:h * 66 + 65],
                                rhs=ptt[:, h, :],
                                start=(kb == 0), stop=(kb == KB - 1))

                    pend = []
                    for kb in range(KB):
                        sps = qkp.tile([128, HPC, 512], f32, tag="sps",
                                       name=f"sps{qc}_{kb}_{b}")
                        for h in range(HPC):
                            nc.tensor.matmul(
                                sps[:, h, :],
                                lhsT=kT_sb[h * 64:(h + 1) * 64,
                                           b * S + kb * 128:
                                           b * S + (kb + 1) * 128],
                                rhs=qT_sb[h * 64:(h + 1) * 64,
                                          b * S + qc * 512:
                                          b * S + (qc + 1) * 512],
                                start=True, stop=True)
                        est = esp.tile([128, HPC, 512], f16, tag="es")
                        nc.scalar.activation(est, sps, func=Exp, scale=SCALE)
                        ptt = ptp.tile([128, HPC, 512], f16, tag="pt")
                        ebt = ebq[:, kb, :].rearrange("p (i q) -> p i q",
                                                      i=HPC)
                        eng = nc.gpsimd if kb % 3 == 2 else nc.vector
                        eng.tensor_mul(ptt, est, ebt)
                        pend.append((ptt, kb))
                        if len(pend) > PIPE:
                            emit_pv(*pend.pop(0))
                    for args in pend:
                        emit_pv(*args)

                    # previous iteration's out-projection: its ctxn inputs
                    # are ready by now, so PE never stalls on it
                    while len(op_pend) > 2:
                        op_pend.pop(0)()

                    # evacuate ctx from PSUM immediately (frees the cv
                    # banks before the broadcast DMA round-trip)
                    cvs = bcp.tile([64, HPC, 512], f32, tag="cvs",
                                   name=f"cvs{qc}_{b}")
                    rc_sb = rcp.tile([65, HPC, 512], f32, tag="rc")
                    for h in range(HPC):
                        nc.vector.reciprocal(rc_sb[64:65, h, :],
                                             cvp[h][64:65, :])
                        nc.vector.tensor_copy(cvs[:, h, :], cvp[h][0:64, :])

                    # normalize: ctxn = ctx.T * (1/den), per h
                    scr = dscr.tile([1, HPC, 512], f32, tag="scr",
                                    name=f"scr{qc}_{b}")
                    nc.sync.dma_start(scr, rc_sb[64:65, :, :])
                    bc = bcp.tile([64, HPC, 512], f32, tag="bc")
                    nc.sync.dma_start(bc, scr.to_broadcast((64, HPC, 512)))
                    nc.vector.tensor_mul(
                        ctxn[b][0:64, qc * 512:(qc + 1) * 512],
                        cvs[:, 0, :], bc[:, 0, :])
                    # h1: lanes 0-63; via ctx1, relocated per qc to
                    # ctxn partitions 64-127
                    nc.vector.tensor_mul(
                        ctx1[b][:, qc * 512:(qc + 1) * 512],
                        cvs[:, 1, :], bc[:, 1, :])
                    nc.sync.dma_start(
                        ctxn[b][64:128, qc * 512:(qc + 1) * 512],
                        ctx1[b][:, qc * 512:(qc + 1) * 512])

                    # out projection for this (qc, b) is emitted one
                    # iteration later (see emit_op) so its matmuls never
                    # wait on the norm chain in the PE FIFO
                    def emit_op(qc=qc, b=b):
                        ob_g = op.tile([128, QC, H], f16, tag="ob",
                                       name=f"ob{qc}_{b}")
                        for ri in range(QC):
                            rb = qc * QC + ri
                            po = qkp.tile([128, HPC, 512], f32, tag="sps",
                                          name=f"po{qc}_{b}_{ri}")
                            lhsT = ctxn[b][:, rb * 128:(rb + 1) * 128]
                            nc.tensor.matmul(po[:, 0, :], lhsT=lhsT,
                                             rhs=wo_sb[:, 0:512],
                                             start=True, stop=True)
                            nc.tensor.matmul(po[:, 1, :], lhsT=lhsT,
                                             rhs=wo_sb[:, 512:1024],
                                             start=True, stop=True)
                            nc.vector.tensor_copy(
                                ob_g[:, ri, :].rearrange("p (i j) -> p i j",
                                                         i=2),
                                po)
                        g0 = b * (S // 128) + qc * QC
                        nc.sync.dma_start(opr[:, g0:g0 + QC, :], ob_g)
                    op_pend.append(emit_op)

            for fn in op_pend:
                fn()


def get_module():
    if "nc" not in _CACHE:
        _CACHE["nc"] = _build_module()
    return _CACHE["nc"]


def make_in_maps(query, key, value, key_padding_mask, bias,
                 q_w, q_b, k_w, k_b, v_w, v_b, o_w, o_b):
    f16 = np.float16
    xq_t = np.ascontiguousarray(query.reshape(R, H).T).astype(f16)
    xk_t = np.ascontiguousarray(key.reshape(R, H).T).astype(f16)
    xv_t = np.ascontiguousarray(value.reshape(R, H).T).astype(f16)

    kpm = np.asarray(key_padding_mask)
    # m01[p, b*?]: column index kb*B + b ; 0.0 where masked
    m01 = np.empty((128, T), np.float32)
    for b in range(B):
        for kb in range(KB):
            m01[:, kb * B + b] = np.where(kpm[b, kb * 128:(kb + 1) * 128],
                                          0.0, 1.0)
    m01_f32 = np.ascontiguousarray(m01)
    # v-order mask: column t = b*KB + kb (matches the v_nat block order)
    m01v = np.empty((128, T), f16)
    for b in range(B):
        for kb in range(KB):
            m01v[:, b * KB + kb] = m01[:, kb * B + b].astype(f16)

    in_maps = []
    for c in range(NCORES):
        hs = slice(c * CW, (c + 1) * CW)
        # eb layout [qc, k, i, qi]: exp(bias).T pre-sliced by q chunk
        ebt = np.empty((QC, S, HPC, 512), f16)
        for i in range(HPC):
            h = c * HPC + i
            e = np.exp(np.asarray(bias[0, h], np.float32).T).astype(f16)
            ebt[:, :, i, :] = e.reshape(S, QC, 512).transpose(1, 0, 2)
        ebt = ebt.reshape(QC, S, HPC * 512)
        in_maps.append({
            "xq_t": xq_t, "xk_t": xk_t, "xv_t": xv_t,
            "wq_t": np.ascontiguousarray(np.asarray(q_w)[hs].T).astype(f16),
            "wk_t": np.ascontiguousarray(np.asarray(k_w)[hs].T).astype(f16),
            "wv_t": np.ascontiguousarray(np.asarray(v_w)[hs].T).astype(f16),
            "wo_t": np.ascontiguousarray(np.asarray(o_w)[:, hs].T).astype(f16),
            "qb_col": np.asarray(q_b, np.float32)[hs].reshape(CW, 1).copy(),
            "kb_col": np.asarray(k_b, np.float32)[hs].reshape(CW, 1).copy(),
            "eb_t": ebt,
            "m01_f32": m01_f32,
            "m01_v": m01v,
        })
    return in_maps


def assemble_output(results, v_b, o_w, o_b):
    acc = np.zeros((R, H), np.float32)
    for res in results:
        acc += np.asarray(res["o_part"], np.float32)
    corr = np.asarray(v_b, np.float32) @ np.asarray(o_w, np.float32).T \
        + np.asarray(o_b, np.float32)
    acc += corr[None, :]
    return acc.reshape(B, S, H).astype(np.float32)


def kernel(**inputs):
    from concourse.bass_utils import run_bass_kernel_spmd

    nc = get_module()
    in_maps = make_in_maps(**inputs)
    res = run_bass_kernel_spmd(nc, in_maps, list(range(NCORES)))
    return assemble_output(res.results, inputs["v_b"], inputs["o_w"],
                           inputs["o_b"])
